# revision 33
# baseline (speedup 1.0000x reference)
"""Trainium2 Bass kernel for nn_CrossAttention (B=2, L=1024, S=2048, DIM=1024, H=16 heads).

Sharding: tensor-parallel over heads x data-parallel over batch.
Core c handles batch b = c//4 and head-group g = c%4 (4 heads = 256 of the
1024 hidden channels).  Each core computes, for its (b, g):

    QT = (Wq_g)^T x_q^T          [256, 1024]   (m on partitions)
    KT = (Wk_g)^T x_k^T          [256, 2048]
    V  = x_v Wv_g                [2048, 256]   (s on partitions)
    per head h (d=64):
        ST_h = KT_h^T' ...       S^T[s, l] = k_s . q_l   (s on partitions)
        P_h  = exp(SCALE * ST_h)            (unnormalized, s on partitions)
        [O^T_h ; sums_h] = [V_h | 1]^T @ P_h   (ones-column folds the softmax
                                                denominator into the matmul)
        XgT_h = O^T_h * (1/sums_h)          (gpsimd partition broadcast)
    out_partial = XgT^T @ Wo_g   [1024, 1024]

Host gathers: out[b] = sum_g out_partial[4b+g] + bo.

Pipeline: the kernel is scalar(exp)-bound in its core, so the structure
minimizes time-to-first-exp and keeps the exp stream dense:
  A: Q projection (2 mt-passes over cached xq chunks, xq DMA issued from the
     scalar queue in parallel with xk on sync), K projection s-half 0, then
     K s-half 1 overlapped with the first 8 ST+exp steps (which only touch
     s-half 0).
  B: V projection (2-bank accumulators, double-pass over cached xv chunks)
     interleaved with the remaining lch0 ST+exp steps.
  C: O(lch0) + ST/exp(lch1).
  D: O(lch1) (accumulators on banks untouched since B, so no false deps on
     C's tail) + Wo(lch0) interleaved.   E: Wo(lch1).
PSUM bank timeline (8 banks): shared(2)+pst(4)+psp(2) in A; shared+pst in B;
shared+pst+ps1(2) in C; shared+wo(4) in D/E.
"""

import sys

if "/opt/trn_rl_repo" not in sys.path:
    sys.path.insert(0, "/opt/trn_rl_repo")

import numpy as np

B, L, S, C = 2, 1024, 2048, 1024
NH, D = 16, 64          # total heads, head dim
HPC = 4                 # heads per core
M = HPC * D             # 256 output channels per core
SCALE = D ** -0.5
P = 128                 # partitions
NCORES = 8
CK = C // P             # 8 c-tiles
NST = S // P            # 16 s-tiles
LCH = 512               # l-chunk
NLCH = L // LCH         # 2

_cache = {}


def _build(debug_dumps=False):
    import concourse.tile as tile
    from concourse import mybir, bacc

    f32 = mybir.dt.float32
    bf16 = mybir.dt.bfloat16

    nc = bacc.Bacc("TRN2", target_bir_lowering=False, debug=False)

    xqT = nc.dram_tensor("xqT", [C, L], bf16, kind="ExternalInput")
    xkT = nc.dram_tensor("xkT", [C, S], bf16, kind="ExternalInput")
    xvT = nc.dram_tensor("xvT", [C, S], bf16, kind="ExternalInput")
    wq = nc.dram_tensor("wq", [C, M], bf16, kind="ExternalInput")
    wk = nc.dram_tensor("wk", [C, M], bf16, kind="ExternalInput")
    wv = nc.dram_tensor("wv", [C, M], bf16, kind="ExternalInput")
    wo = nc.dram_tensor("wo", [M, C], bf16, kind="ExternalInput")
    outp = nc.dram_tensor("outp", [L, C], bf16, kind="ExternalOutput")
    if debug_dumps:
        dbg_qt = nc.dram_tensor("dbg_qt", [P, 2, L], bf16, kind="ExternalOutput")
        dbg_kt = nc.dram_tensor("dbg_kt", [P, 2, S], bf16, kind="ExternalOutput")
        dbg_vones = nc.dram_tensor("dbg_vones", [P, NST, HPC, D + 1], bf16,
                                   kind="ExternalOutput")
        dbg_pt = nc.dram_tensor("dbg_pt", [P, 2, LCH], bf16, kind="ExternalOutput")
        dbg_rc = nc.dram_tensor("dbg_rc", [1, LCH], f32, kind="ExternalOutput")
        dbg_bc = nc.dram_tensor("dbg_bc", [D, LCH], f32, kind="ExternalOutput")
        dbg_xgt = nc.dram_tensor("dbg_xgt", [P, 2, L], bf16, kind="ExternalOutput")

    with tile.TileContext(nc) as tc:
        with tc.tile_pool(name="singles", bufs=1) as singles, \
             tc.tile_pool(name="xq_pool", bufs=8) as xqp, \
             tc.tile_pool(name="xk_pool", bufs=16) as xkp, \
             tc.tile_pool(name="xv_pool", bufs=16) as xvp, \
             tc.tile_pool(name="pts", bufs=34) as pts, \
             tc.tile_pool(name="small", bufs=4) as small, \
             tc.tile_pool(name="obuf", bufs=8) as obuf:

            # ---- persistent SBUF ----
            wq_sb = singles.tile([P, CK, M], bf16, tag="wq")
            wk_sb = singles.tile([P, CK, M], bf16, tag="wk")
            wv_sb = singles.tile([P, CK, M], bf16, tag="wv")
            wo_sb = singles.tile([P, M // P, C], bf16, tag="wo")
            nc.sync.dma_start(wq_sb[:], wq.rearrange("(ck p) m -> p ck m", p=P))

            kt_sb = singles.tile([P, 2, S], bf16, tag="kt")        # [m%128, m//128, s]
            qt_sb = singles.tile([P, 2, L], bf16, tag="qt")        # [m%128, m//128, l]
            vones = singles.tile([P, NST, HPC, D + 1], bf16, tag="vones")
            xgt_sb = singles.tile([P, 2, L], bf16, tag="xgt")
            stage = singles.tile([P, D], f32, tag="stage")
            nc.vector.memset(stage[:], 1.0)
            nc.vector.tensor_copy(vones[:, :, :, D:D + 1],
                                  stage[:].rearrange("p (a b) -> p a b", a=NST)[:, :, :, None])

            # ---- step helpers ----
            def st_step(lch, pair, st):
                """ST pair matmuls + exp; returns the PT tile."""
                lsl = slice(lch * LCH, (lch + 1) * LCH)
                ssl = slice(st * P, (st + 1) * P)
                st_ps = pst.tile([P, 2, LCH], f32, tag="st", name=f"stps_{lch}_{pair}_{st}")
                nc.tensor.matmul(
                    st_ps[:, 0, :], kt_sb[0:D, pair, ssl], qt_sb[0:D, pair, lsl],
                    start=True, stop=True)
                nc.tensor.matmul(
                    st_ps[:, 1, :], kt_sb[D:P, pair, ssl], qt_sb[D:P, pair, lsl],
                    start=True, stop=True, tile_position=(64, 0))
                pt_t = pts.tile([P, 2, LCH], bf16, tag="pt", name=f"pt_{lch}_{pair}_{st}")
                nc.scalar.activation(pt_t[:], st_ps[:],
                                     mybir.ActivationFunctionType.Exp, scale=SCALE)
                if debug_dumps and lch == 0 and pair == 0 and st == 0:
                    nc.sync.dma_start(dbg_pt[:], pt_t[:])
                return pt_t

            def o_step(o_ps, lch, pair, st, pt_t):
                for hh in range(2):
                    nc.tensor.matmul(
                        o_ps[hh][:], vones[:, st, pair * 2 + hh, :], pt_t[:, hh, :],
                        start=(st == 0), stop=(st == NST - 1))

            def norm_pair(lch, pair, o_ps):
                """fast reciprocal of sums row -> gpsimd partition broadcast
                -> normalized XgT (no PSUM bank, no tensor-engine matmul)."""
                lsl = slice(lch * LCH, (lch + 1) * LCH)
                for hh in range(2):
                    # rc lives at partition 0: the gpsimd broadcast firmware
                    # reads the source on Q7 core 0, which only sees
                    # partitions 0-15.  Stage the PSUM sums row into SBUF
                    # first (custom-DVE bit ops need an SBUF source).
                    sums_sb = small.tile([1, LCH], f32, tag="sums")
                    nc.vector.tensor_copy(sums_sb[:], o_ps[hh][D:D + 1, :])
                    rc = small.tile([1, LCH], f32, tag="rc")
                    nc.vector.reciprocal_approx_fast(rc[:], sums_sb[:])
                    bc_sb = small.tile([D, LCH], f32, tag="bc")
                    nc.gpsimd.partition_broadcast(bc_sb[:], rc[:])
                    if debug_dumps and lch == 0 and pair == 0 and hh == 0:
                        nc.sync.dma_start(dbg_rc[:], rc[:])
                        nc.sync.dma_start(dbg_bc[:], bc_sb[:])
                    nc.vector.tensor_mul(
                        xgt_sb[hh * D:(hh + 1) * D, pair, lsl],
                        o_ps[hh][0:D, :], bc_sb[:])

            def wo_step(pool, lt, nch, cast_eng):
                wo_ps = pool.tile([P, 512], f32, tag="wo", name=f"wops_{lt}_{nch}")
                for kt in range(2):
                    nc.tensor.matmul(
                        wo_ps[:], xgt_sb[:, kt, lt * P:(lt + 1) * P],
                        wo_sb[:, kt, nch * 512:(nch + 1) * 512],
                        start=(kt == 0), stop=(kt == 1))
                ob_sb = obuf.tile([P, 512], bf16, tag="ob")
                if cast_eng == "scalar":
                    nc.scalar.copy(ob_sb[:], wo_ps[:])
                    nc.scalar.dma_start(
                        outp[lt * P:(lt + 1) * P, nch * 512:(nch + 1) * 512], ob_sb[:])
                else:
                    nc.vector.tensor_copy(ob_sb[:], wo_ps[:])
                    nc.gpsimd.dma_start(
                        outp[lt * P:(lt + 1) * P, nch * 512:(nch + 1) * 512], ob_sb[:])

            # ---- PSUM pool timeline (LIFO):
            #   shared(2) > pst(4) > [psp(2) A] > [ps1(2) C] > close pst >
            #   [wo(4) D/E] > close shared
            shared_cm = tc.tile_pool(name="ps_shared", bufs=2, space="PSUM")
            shared = shared_cm.__enter__()
            pst_cm = tc.tile_pool(name="ps_st", bufs=2, space="PSUM")
            pst = pst_cm.__enter__()

            pt0 = {}   # (pair, st) -> PT tile for lch 0
            pt1 = {}
            SH = S // 2

            # =========== Phase A: QT + KT projections ===========
            with tc.tile_pool(name="ps_proj", bufs=2, space="PSUM") as psp:
                # xq chunk DMAs issue from the scalar queue, in parallel with
                # wq/wk/xk on sync.
                xq_ts = []
                for ck in range(CK):
                    xq_t = xqp.tile([P, L], bf16, tag="xq")
                    nc.sync.dma_start(xq_t[:], xqT[ck * P:(ck + 1) * P, :])
                    xq_ts.append(xq_t)
                nc.sync.dma_start(wk_sb[:], wk.rearrange("(ck p) m -> p ck m", p=P))
                xk_ts = []
                for ck in range(CK):
                    xk_t = xkp.tile([P, SH], bf16, tag="xk")
                    nc.sync.dma_start(xk_t[:], xkT[ck * P:(ck + 1) * P, 0:SH])
                    xk_ts.append(xk_t)
                xk1_ts = []
                for ck in range(CK):
                    xk_t = xkp.tile([P, SH], bf16, tag="xk")
                    nc.sync.dma_start(xk_t[:], xkT[ck * P:(ck + 1) * P, SH:S])
                    xk1_ts.append(xk_t)
                nc.sync.dma_start(wv_sb[:], wv.rearrange("(ck p) m -> p ck m", p=P))
                nc.sync.dma_start(wo_sb[:], wo.rearrange("(kt p) n -> p kt n", p=P))

                # --- mt0 (= head-pair 0) passes first: Q-mt0, K-sh0-mt0,
                #     then K-sh1-mt0 / Q-mt1 / K-sh0-mt1 / K-sh1-mt1 each
                #     interleaved with 8 ST+exp steps that just became
                #     runnable.  pair == mt, so pair-0 STs need only the
                #     mt0 projections. ---
                def q_pass(mt, st_jobs):
                    q_ps = [psp.tile([P, 512], f32, tag="pp", name=f"qtps{mt}_{lh}")
                            for lh in range(2)]
                    for ck in range(CK):
                        for lh in range(2):
                            nc.tensor.matmul(
                                q_ps[lh][:],
                                wq_sb[:, ck, mt * P:(mt + 1) * P],
                                xq_ts[ck][:, lh * 512:(lh + 1) * 512],
                                start=(ck == 0), stop=(ck == CK - 1))
                        if st_jobs:
                            pair, st = st_jobs.pop(0)
                            pt0[(pair, st)] = st_step(0, pair, st)
                    for lh in range(2):
                        nc.vector.tensor_copy(
                            qt_sb[:, mt, lh * 512:(lh + 1) * 512], q_ps[lh][:])

                def k_pass(mt, sh, chunks, st_jobs):
                    k_ps = [psp.tile([P, 512], f32, tag="pp",
                                     name=f"ktps{sh}_{mt}_{nh}") for nh in range(2)]
                    for ck in range(CK):
                        for nh in range(2):
                            nc.tensor.matmul(
                                k_ps[nh][:],
                                wk_sb[:, ck, mt * P:(mt + 1) * P],
                                chunks[ck][:, nh * 512:(nh + 1) * 512],
                                start=(ck == 0), stop=(ck == CK - 1))
                        if st_jobs:
                            pair, st = st_jobs.pop(0)
                            pt0[(pair, st)] = st_step(0, pair, st)
                    for nh in range(2):
                        nc.vector.tensor_copy(
                            kt_sb[:, mt, sh * SH + nh * 512:sh * SH + (nh + 1) * 512],
                            k_ps[nh][:])

                q_pass(0, [])
                k_pass(0, 0, xk_ts, [])
                # first 8 ST+exp steps run standalone while xk s-half 1 lands
                for st in range(8):
                    pt0[(0, st)] = st_step(0, 0, st)
                k_pass(0, 1, xk1_ts, [])
                q_pass(1, [(0, st) for st in range(8, 16)])
                k_pass(1, 0, xk_ts, [])
                k_pass(1, 1, xk1_ts, [(1, st) for st in range(8)])

            # =========== Phase B: V projection, O(lch0,p0), lch0-p1 +
            #             lch1-p0 ST/exp streams ===========
            # xv streams from the gpsimd queue, but only after phase A's
            # critical input (gate on the 6th xk s-half-1 chunk).
            gate_sb = singles.tile([1, 2], bf16, tag="gate")
            nc.gpsimd.tensor_copy(gate_sb[:], xk1_ts[5][0:1, 0:2])

            st_jobs = [(0, 1, st) for st in range(8, 16)] + \
                      [(1, 0, st) for st in range(16)]
            psB_cm = tc.tile_pool(name="ps_b", bufs=2, space="PSUM")
            psB = psB_cm.__enter__()
            oB = [psB.tile([D + 1, LCH], f32, tag="ob", name=f"oB_{i}")
                  for i in range(2)]
            for q in range(4):
                xv_ts = []
                for half in range(2):
                    v_ps = [shared.tile([P, M], f32, tag="sh",
                                        name=f"vps{q}_{half}_{i}")
                            for i in range(2)]
                    for ck in range(CK):
                        if half == 0:
                            xv_t = xvp.tile([P, 4 * P], bf16, tag="xv")
                            nc.gpsimd.dma_start(
                                xv_t[:], xvT[ck * P:(ck + 1) * P,
                                             q * 4 * P:(q + 1) * 4 * P])
                            xv_ts.append(xv_t)
                        else:
                            xv_t = xv_ts[ck]
                        for st2 in range(2):
                            st4 = half * 2 + st2
                            nc.tensor.matmul(
                                v_ps[st2][:],
                                xv_t[:, st4 * P:(st4 + 1) * P],
                                wv_sb[:, ck, :],
                                start=(ck == 0), stop=(ck == CK - 1))
                        if ck % 2 == half and st_jobs:
                            lch, pair, st = st_jobs.pop(0)
                            pt = st_step(lch, pair, st)
                            (pt0 if lch == 0 else pt1)[(pair, st)] = pt
                    for st2 in range(2):
                        st = q * 4 + half * 2 + st2
                        nc.vector.tensor_copy(
                            vones[:, st, :, 0:D],
                            v_ps[st2][:].rearrange("p (h d) -> p h d", h=HPC))
                # O(lch0, pair0) over the previous quarter's s-tiles
                for st in range(max(0, 4 * q - 4), 4 * q):
                    o_step(oB, 0, 0, st, pt0.pop((0, st)))
            for st in range(12, 16):
                o_step(oB, 0, 0, st, pt0.pop((0, st)))
            norm_pair(0, 0, oB)
            psB_cm.__exit__(None, None, None)

            # =========== Phase C: O(lch0,p1) + O(lch1,p0) + lch1-p1 ST ====
            with tc.tile_pool(name="ps_c", bufs=2, space="PSUM") as ps1:
                oC1 = [ps1.tile([D + 1, LCH], f32, tag="ps1",
                                name=f"oC1_{i}") for i in range(2)]
                oC2 = [shared.tile([D + 1, LCH], f32, tag="sh",
                                   name=f"oC2_{i}") for i in range(2)]
                for st in range(NST):
                    o_step(oC1, 0, 1, st, pt0.pop((1, st)))
                    o_step(oC2, 1, 0, st, pt1.pop((0, st)))
                    pt1[(1, st)] = st_step(1, 1, st)
                norm_pair(0, 1, oC1)
                norm_pair(1, 0, oC2)

            pst_cm.__exit__(None, None, None)

            # =========== Phases D+E: O(lch1,p1) + Wo ===========
            with tc.tile_pool(name="ps_wo", bufs=3, space="PSUM") as pswo:
                wo_jobs0 = [(lt, nch) for lt in range(4) for nch in range(2)]
                ncast = 0
                oD = [pswo.tile([D + 1, LCH], f32, tag="psd",
                                name=f"oD_{i}") for i in range(2)]
                for st in range(NST):
                    o_step(oD, 1, 1, st, pt1.pop((1, st)))
                    if st % 2 == 1 and wo_jobs0:
                        wo_step(pswo, *wo_jobs0.pop(0),
                                "scalar" if ncast % 2 == 0 else "vector")
                        ncast += 1
                norm_pair(1, 1, oD)

                for lt in range(4, 8):
                    for nch in range(2):
                        wo_step(pswo, lt, nch,
                                "scalar" if ncast % 2 == 0 else "vector")
                        ncast += 1

                if debug_dumps:
                    nc.sync.dma_start(dbg_qt[:], qt_sb[:])
                    nc.sync.dma_start(dbg_kt[:], kt_sb[:])
                    nc.sync.dma_start(dbg_vones[:], vones[:])
                    nc.sync.dma_start(dbg_xgt[:], xgt_sb[:])

            shared_cm.__exit__(None, None, None)

    nc.compile()
    return nc


def _get_nc():
    if "nc" not in _cache:
        _cache["nc"] = _build()
    return _cache["nc"]


def _make_in_maps(inputs):
    import ml_dtypes

    bf16 = ml_dtypes.bfloat16
    query = np.asarray(inputs["query"], dtype=np.float32)
    key = np.asarray(inputs["key"], dtype=np.float32)
    value = np.asarray(inputs["value"], dtype=np.float32)
    Wq = np.asarray(inputs["Wq"], dtype=np.float32)
    Wk = np.asarray(inputs["Wk"], dtype=np.float32)
    Wv = np.asarray(inputs["Wv"], dtype=np.float32)
    Wo = np.asarray(inputs["Wo"], dtype=np.float32)

    qT = [np.ascontiguousarray(query[b].T).astype(bf16) for b in range(B)]
    kT = [np.ascontiguousarray(key[b].T).astype(bf16) for b in range(B)]
    vT = [np.ascontiguousarray(value[b].T).astype(bf16) for b in range(B)]
    wq_s = [np.ascontiguousarray(Wq[:, g * M:(g + 1) * M]).astype(bf16) for g in range(4)]
    wk_s = [np.ascontiguousarray(Wk[:, g * M:(g + 1) * M]).astype(bf16) for g in range(4)]
    wv_s = [np.ascontiguousarray(Wv[:, g * M:(g + 1) * M]).astype(bf16) for g in range(4)]
    wo_s = [np.ascontiguousarray(Wo[g * M:(g + 1) * M, :]).astype(bf16) for g in range(4)]

    in_maps = []
    for core in range(NCORES):
        b, g = core // 4, core % 4
        in_maps.append({
            "xqT": qT[b], "xkT": kT[b], "xvT": vT[b],
            "wq": wq_s[g], "wk": wk_s[g], "wv": wv_s[g], "wo": wo_s[g],
        })
    return in_maps


def kernel(query, key, value, Wq, Wk, Wv, Wo, bo):
    from concourse.bass_utils import run_bass_kernel_spmd

    nc = _get_nc()
    bo = np.asarray(bo, dtype=np.float32)
    in_maps = _make_in_maps(dict(query=query, key=key, value=value,
                                 Wq=Wq, Wk=Wk, Wv=Wv, Wo=Wo))

    res = run_bass_kernel_spmd(nc, in_maps, core_ids=list(range(NCORES)))

    out = np.zeros((B, L, C), dtype=np.float32)
    for core in range(NCORES):
        b = core // 4
        out[b] += np.asarray(res.results[core]["outp"], dtype=np.float32)
    out += bo[None, None, :]
    return out


# revision 34
# speedup vs baseline: 1.0778x; 1.0778x over previous
"""Trainium2 Bass kernel for nn_CrossAttention (B=2, L=1024, S=2048, DIM=1024, H=16 heads).

Sharding: tensor-parallel over heads x data-parallel over batch.
Core c handles batch b = c//4 and head-group g = c%4 (4 heads = 256 of the
1024 hidden channels).  Each core computes, for its (b, g):

    QT = (Wq_g)^T x_q^T          [256, 1024]   (m on partitions)
    KT = (Wk_g)^T x_k^T          [256, 2048]
    V  = x_v Wv_g                [2048, 256]   (s on partitions)
    per head h (d=64):
        ST_h = KT_h^T' ...       S^T[s, l] = k_s . q_l   (s on partitions)
        P_h  = exp(SCALE * ST_h)            (unnormalized, s on partitions)
        [O^T_h ; sums_h] = [V_h | 1]^T @ P_h   (ones-column folds the softmax
                                                denominator into the matmul)
        XgT_h = O^T_h * (1/sums_h)          (gpsimd partition broadcast)
    out_partial = XgT^T @ Wo_g   [1024, 1024]

Host gathers: out[b] = sum_g out_partial[4b+g] + bo.

The kernel is scalar(exp)-bound in its core, so the structure minimizes
time-to-first-exp and keeps the 64-exp stream dense:
  A: xq/xk land as ONE strided DMA each (full 16-engine striping, no
     per-chunk issue pacing).  Projections run as 2-bank double passes
     (Q-mt0, K-sh0 both mt, then the first 8 ST+exp steps standalone while
     K-sh1 passes run).  PSUM->SBUF casts on the vector engine, off the
     scalar stream.
  B: V projection (2-bank accumulators, double-pass over cached xv chunks)
     interleaved with the remaining 24 lch0 ST+exp steps.
  C: O(lch0) + ST/exp(lch1).
  D: O(lch1) (PSUM banks from the shared ring, no false deps on C's tail)
     + Wo(lch0) interleaved.   E: Wo(lch1).
Output partials are written bf16 (host accumulates fp32); the softmax
reciprocal uses the fast approximate DVE op + gpsimd partition broadcast.
"""

import sys

if "/opt/trn_rl_repo" not in sys.path:
    sys.path.insert(0, "/opt/trn_rl_repo")

import numpy as np

B, L, S, C = 2, 1024, 2048, 1024
NH, D = 16, 64          # total heads, head dim
HPC = 4                 # heads per core
M = HPC * D             # 256 output channels per core
SCALE = D ** -0.5
P = 128                 # partitions
NCORES = 8
CK = C // P             # 8 c-tiles
NST = S // P            # 16 s-tiles
LCH = 512               # l-chunk
NLCH = L // LCH         # 2

_cache = {}


def _build(debug_dumps=False):
    import concourse.tile as tile
    from concourse import mybir, bacc

    f32 = mybir.dt.float32
    bf16 = mybir.dt.bfloat16

    nc = bacc.Bacc("TRN2", target_bir_lowering=False, debug=False)

    xqT = nc.dram_tensor("xqT", [C, L], bf16, kind="ExternalInput")
    xkT = nc.dram_tensor("xkT", [C, S], bf16, kind="ExternalInput")
    xvT = nc.dram_tensor("xvT", [C, S], bf16, kind="ExternalInput")
    wq = nc.dram_tensor("wq", [C, M], bf16, kind="ExternalInput")
    wk = nc.dram_tensor("wk", [C, M], bf16, kind="ExternalInput")
    wv = nc.dram_tensor("wv", [C, M], bf16, kind="ExternalInput")
    wo = nc.dram_tensor("wo", [M, C], bf16, kind="ExternalInput")
    outp = nc.dram_tensor("outp", [L, C], bf16, kind="ExternalOutput")
    if debug_dumps:
        dbg_qt = nc.dram_tensor("dbg_qt", [P, 2, L], bf16, kind="ExternalOutput")
        dbg_kt = nc.dram_tensor("dbg_kt", [P, 2, S], bf16, kind="ExternalOutput")
        dbg_vones = nc.dram_tensor("dbg_vones", [P, NST, HPC, D + 1], bf16,
                                   kind="ExternalOutput")
        dbg_pt = nc.dram_tensor("dbg_pt", [P, 2, LCH], bf16, kind="ExternalOutput")
        dbg_rc = nc.dram_tensor("dbg_rc", [1, LCH], f32, kind="ExternalOutput")
        dbg_bc = nc.dram_tensor("dbg_bc", [D, LCH], f32, kind="ExternalOutput")
        dbg_xgt = nc.dram_tensor("dbg_xgt", [P, 2, L], bf16, kind="ExternalOutput")

    with tile.TileContext(nc) as tc:
        with tc.tile_pool(name="singles", bufs=1) as singles, \
             tc.tile_pool(name="xv_pool", bufs=16) as xvp, \
             tc.tile_pool(name="pts", bufs=34) as pts, \
             tc.tile_pool(name="small", bufs=4) as small, \
             tc.tile_pool(name="obuf", bufs=8) as obuf:

            # ---- persistent SBUF ----
            wq_sb = singles.tile([P, CK, M], bf16, tag="wq")
            wk_sb = singles.tile([P, CK, M], bf16, tag="wk")
            wv_sb = singles.tile([P, CK, M], bf16, tag="wv")
            wo_sb = singles.tile([P, M // P, C], bf16, tag="wo")
            xq_sb = singles.tile([P, CK, L], bf16, tag="xq")
            xk_sb = singles.tile([P, CK, S], bf16, tag="xkc")
            SH = S // 2
            # single big strided DMAs: 16-engine striping from issue #1,
            # no per-chunk sync-queue issue pacing.
            nc.sync.dma_start(wq_sb[:], wq.rearrange("(ck p) m -> p ck m", p=P))
            nc.sync.dma_start(xq_sb[:], xqT.rearrange("(ck p) l -> p ck l", p=P))
            nc.sync.dma_start(wk_sb[:], wk.rearrange("(ck p) m -> p ck m", p=P))
            nc.sync.dma_start(xk_sb[:, :, 0:SH],
                              xkT[:, 0:SH].rearrange("(ck p) s -> p ck s", p=P))
            nc.sync.dma_start(xk_sb[:, :, SH:S],
                              xkT[:, SH:S].rearrange("(ck p) s -> p ck s", p=P))
            nc.sync.dma_start(wv_sb[:], wv.rearrange("(ck p) m -> p ck m", p=P))
            nc.sync.dma_start(wo_sb[:], wo.rearrange("(kt p) n -> p kt n", p=P))

            kt_sb = singles.tile([P, 2, S], bf16, tag="kt")        # [m%128, m//128, s]
            qt_sb = singles.tile([P, 2, L], bf16, tag="qt")        # [m%128, m//128, l]
            vones = singles.tile([P, NST, HPC, D + 1], bf16, tag="vones")
            xgt_sb = singles.tile([P, 2, L], bf16, tag="xgt")
            stage = singles.tile([P, D], f32, tag="stage")
            nc.vector.memset(stage[:], 1.0)
            nc.vector.tensor_copy(vones[:, :, :, D:D + 1],
                                  stage[:].rearrange("p (a b) -> p a b", a=NST)[:, :, :, None])

            # ---- step helpers ----
            def st_step(lch, pair, st):
                """ST pair matmuls + exp; returns the PT tile."""
                lsl = slice(lch * LCH, (lch + 1) * LCH)
                ssl = slice(st * P, (st + 1) * P)
                st_ps = pst.tile([P, 2, LCH], f32, tag="st", name=f"stps_{lch}_{pair}_{st}")
                nc.tensor.matmul(
                    st_ps[:, 0, :], kt_sb[0:D, pair, ssl], qt_sb[0:D, pair, lsl],
                    start=True, stop=True)
                nc.tensor.matmul(
                    st_ps[:, 1, :], kt_sb[D:P, pair, ssl], qt_sb[D:P, pair, lsl],
                    start=True, stop=True, tile_position=(64, 0))
                pt_t = pts.tile([P, 2, LCH], bf16, tag="pt", name=f"pt_{lch}_{pair}_{st}")
                nc.scalar.activation(pt_t[:], st_ps[:],
                                     mybir.ActivationFunctionType.Exp, scale=SCALE)
                if debug_dumps and lch == 0 and pair == 0 and st == 0:
                    nc.sync.dma_start(dbg_pt[:], pt_t[:])
                return pt_t

            def o_step(o_ps, lch, pair, st, pt_t):
                for hh in range(2):
                    nc.tensor.matmul(
                        o_ps[hh][:], vones[:, st, pair * 2 + hh, :], pt_t[:, hh, :],
                        start=(st == 0), stop=(st == NST - 1))

            def norm_pair(lch, pair, o_ps):
                """fast reciprocal of sums row -> gpsimd partition broadcast
                -> normalized XgT (no PSUM bank, no tensor-engine matmul)."""
                lsl = slice(lch * LCH, (lch + 1) * LCH)
                for hh in range(2):
                    # rc lives at partition 0: the gpsimd broadcast firmware
                    # reads the source on Q7 core 0, which only sees
                    # partitions 0-15.  Stage the PSUM sums row into SBUF
                    # first (custom-DVE bit ops need an SBUF source).
                    sums_sb = small.tile([1, LCH], f32, tag="sums")
                    nc.vector.tensor_copy(sums_sb[:], o_ps[hh][D:D + 1, :])
                    rc = small.tile([1, LCH], f32, tag="rc")
                    nc.vector.reciprocal_approx_fast(rc[:], sums_sb[:])
                    bc_sb = small.tile([D, LCH], f32, tag="bc")
                    nc.gpsimd.partition_broadcast(bc_sb[:], rc[:])
                    if debug_dumps and lch == 0 and pair == 0 and hh == 0:
                        nc.sync.dma_start(dbg_rc[:], rc[:])
                        nc.sync.dma_start(dbg_bc[:], bc_sb[:])
                    nc.vector.tensor_mul(
                        xgt_sb[hh * D:(hh + 1) * D, pair, lsl],
                        o_ps[hh][0:D, :], bc_sb[:])

            def wo_step(pool, lt, nch, cast_eng):
                wo_ps = pool.tile([P, 512], f32, tag="wo", name=f"wops_{lt}_{nch}")
                for kt in range(2):
                    nc.tensor.matmul(
                        wo_ps[:], xgt_sb[:, kt, lt * P:(lt + 1) * P],
                        wo_sb[:, kt, nch * 512:(nch + 1) * 512],
                        start=(kt == 0), stop=(kt == 1))
                ob_sb = obuf.tile([P, 512], bf16, tag="ob")
                if cast_eng == "scalar":
                    nc.scalar.copy(ob_sb[:], wo_ps[:])
                    nc.scalar.dma_start(
                        outp[lt * P:(lt + 1) * P, nch * 512:(nch + 1) * 512], ob_sb[:])
                else:
                    nc.vector.tensor_copy(ob_sb[:], wo_ps[:])
                    nc.gpsimd.dma_start(
                        outp[lt * P:(lt + 1) * P, nch * 512:(nch + 1) * 512], ob_sb[:])

            # ---- PSUM pool timeline (LIFO):
            #   shared(2) > pst(4) > [psp(2) A] > [ps1(2) C] > close pst >
            #   [wo(4) D/E] > close shared
            shared_cm = tc.tile_pool(name="ps_shared", bufs=2, space="PSUM")
            shared = shared_cm.__enter__()
            pst_cm = tc.tile_pool(name="ps_st", bufs=2, space="PSUM")
            pst = pst_cm.__enter__()

            pt0 = {}   # (pair, st) -> PT tile for lch 0
            pt1 = {}

            # =========== Phase A: QT + KT projections ===========
            with tc.tile_pool(name="ps_proj", bufs=2, space="PSUM") as psp:

                def q_pass(mt):
                    q_ps = [psp.tile([P, 512], f32, tag="pp", name=f"qtps{mt}_{lh}")
                            for lh in range(2)]
                    for ck in range(CK):
                        for lh in range(2):
                            nc.tensor.matmul(
                                q_ps[lh][:],
                                wq_sb[:, ck, mt * P:(mt + 1) * P],
                                xq_sb[:, ck, lh * 512:(lh + 1) * 512],
                                start=(ck == 0), stop=(ck == CK - 1))
                    for lh in range(2):
                        nc.vector.tensor_copy(
                            qt_sb[:, mt, lh * 512:(lh + 1) * 512], q_ps[lh][:])

                def k_pass(mt, sh):
                    k_ps = [psp.tile([P, 512], f32, tag="pp",
                                     name=f"ktps{sh}_{mt}_{nh}") for nh in range(2)]
                    for ck in range(CK):
                        for nh in range(2):
                            nc.tensor.matmul(
                                k_ps[nh][:],
                                wk_sb[:, ck, mt * P:(mt + 1) * P],
                                xk_sb[:, ck, sh * SH + nh * 512:sh * SH + (nh + 1) * 512],
                                start=(ck == 0), stop=(ck == CK - 1))
                    for nh in range(2):
                        nc.vector.tensor_copy(
                            kt_sb[:, mt, sh * SH + nh * 512:sh * SH + (nh + 1) * 512],
                            k_ps[nh][:])

                q_pass(0)
                k_pass(0, 0)
                # first 8 ST+exp steps run standalone: they only need the
                # mt0 projections and KT s-half 0 (s-tiles 0-7).
                for st in range(8):
                    pt0[(0, st)] = st_step(0, 0, st)
                k_pass(0, 1)
                q_pass(1)
                k_pass(1, 0)
                k_pass(1, 1)

            # =========== Phase B: V projection + remaining lch0 ST steps ====
            step = CK  # pair-0 s-tiles 0-7 ran in phase A
            for q in range(4):
                xv_ts = []
                for half in range(2):
                    v_ps = [shared.tile([P, M], f32, tag="sh",
                                        name=f"vps{q}_{half}_{i}")
                            for i in range(2)]
                    for ck in range(CK):
                        if half == 0:
                            xv_t = xvp.tile([P, 4 * P], bf16, tag="xv")
                            nc.sync.dma_start(
                                xv_t[:], xvT[ck * P:(ck + 1) * P,
                                             q * 4 * P:(q + 1) * 4 * P])
                            xv_ts.append(xv_t)
                        else:
                            xv_t = xv_ts[ck]
                        for st2 in range(2):
                            st4 = half * 2 + st2
                            nc.tensor.matmul(
                                v_ps[st2][:],
                                xv_t[:, st4 * P:(st4 + 1) * P],
                                wv_sb[:, ck, :],
                                start=(ck == 0), stop=(ck == CK - 1))
                        if ck % 2 == half and step < 2 * NST:
                            pair, st = divmod(step, NST)
                            pt0[(pair, st)] = st_step(0, pair, st)
                            step += 1
                    for st2 in range(2):
                        st = q * 4 + half * 2 + st2
                        nc.vector.tensor_copy(
                            vones[:, st, :, 0:D],
                            v_ps[st2][:].rearrange("p (h d) -> p h d", h=HPC))

            # =========== Phase C: O(lch0) + lch1 ST/exp ===========
            with tc.tile_pool(name="ps_c", bufs=2, space="PSUM") as ps1:
                for pair in range(2):
                    o_ps = [ps1.tile([D + 1, LCH], f32, tag="ps1",
                                     name=f"ops0_{pair}_{i}") for i in range(2)]
                    for st in range(NST):
                        o_step(o_ps, 0, pair, st, pt0.pop((pair, st)))
                        pt1[(pair, st)] = st_step(1, pair, st)
                    norm_pair(0, pair, o_ps)

            pst_cm.__exit__(None, None, None)

            # =========== Phases D+E: O(lch1) + Wo ===========
            with tc.tile_pool(name="ps_wo", bufs=4, space="PSUM") as pswo:
                wo_jobs0 = [(lt, nch) for lt in range(4) for nch in range(2)]
                ncast = 0
                for pair in range(2):
                    o_ps = [shared.tile([D + 1, LCH], f32, tag="sh",
                                        name=f"ops1_{pair}_{i}") for i in range(2)]
                    for st in range(NST):
                        o_step(o_ps, 1, pair, st, pt1.pop((pair, st)))
                        if st % 4 == 1 and wo_jobs0:
                            wo_step(pswo, *wo_jobs0.pop(0),
                                    "scalar" if ncast % 2 == 0 else "vector")
                            ncast += 1
                    norm_pair(1, pair, o_ps)
                for lt, nch in wo_jobs0:
                    wo_step(pswo, lt, nch, "scalar" if ncast % 2 == 0 else "vector")
                    ncast += 1

                for lt in range(4, 8):
                    for nch in range(2):
                        wo_step(pswo, lt, nch,
                                "scalar" if ncast % 2 == 0 else "vector")
                        ncast += 1

                if debug_dumps:
                    nc.sync.dma_start(dbg_qt[:], qt_sb[:])
                    nc.sync.dma_start(dbg_kt[:], kt_sb[:])
                    nc.sync.dma_start(dbg_vones[:], vones[:])
                    nc.sync.dma_start(dbg_xgt[:], xgt_sb[:])

            shared_cm.__exit__(None, None, None)

    nc.compile()
    return nc


def _get_nc():
    if "nc" not in _cache:
        _cache["nc"] = _build()
    return _cache["nc"]


def _make_in_maps(inputs):
    import ml_dtypes

    bf16 = ml_dtypes.bfloat16
    query = np.asarray(inputs["query"], dtype=np.float32)
    key = np.asarray(inputs["key"], dtype=np.float32)
    value = np.asarray(inputs["value"], dtype=np.float32)
    Wq = np.asarray(inputs["Wq"], dtype=np.float32)
    Wk = np.asarray(inputs["Wk"], dtype=np.float32)
    Wv = np.asarray(inputs["Wv"], dtype=np.float32)
    Wo = np.asarray(inputs["Wo"], dtype=np.float32)

    qT = [np.ascontiguousarray(query[b].T).astype(bf16) for b in range(B)]
    kT = [np.ascontiguousarray(key[b].T).astype(bf16) for b in range(B)]
    vT = [np.ascontiguousarray(value[b].T).astype(bf16) for b in range(B)]
    wq_s = [np.ascontiguousarray(Wq[:, g * M:(g + 1) * M]).astype(bf16) for g in range(4)]
    wk_s = [np.ascontiguousarray(Wk[:, g * M:(g + 1) * M]).astype(bf16) for g in range(4)]
    wv_s = [np.ascontiguousarray(Wv[:, g * M:(g + 1) * M]).astype(bf16) for g in range(4)]
    wo_s = [np.ascontiguousarray(Wo[g * M:(g + 1) * M, :]).astype(bf16) for g in range(4)]

    in_maps = []
    for core in range(NCORES):
        b, g = core // 4, core % 4
        in_maps.append({
            "xqT": qT[b], "xkT": kT[b], "xvT": vT[b],
            "wq": wq_s[g], "wk": wk_s[g], "wv": wv_s[g], "wo": wo_s[g],
        })
    return in_maps


def kernel(query, key, value, Wq, Wk, Wv, Wo, bo):
    from concourse.bass_utils import run_bass_kernel_spmd

    nc = _get_nc()
    bo = np.asarray(bo, dtype=np.float32)
    in_maps = _make_in_maps(dict(query=query, key=key, value=value,
                                 Wq=Wq, Wk=Wk, Wv=Wv, Wo=Wo))

    res = run_bass_kernel_spmd(nc, in_maps, core_ids=list(range(NCORES)))

    out = np.zeros((B, L, C), dtype=np.float32)
    for core in range(NCORES):
        b = core // 4
        out[b] += np.asarray(res.results[core]["outp"], dtype=np.float32)
    out += bo[None, None, :]
    return out


# revision 35
# speedup vs baseline: 1.1004x; 1.0210x over previous
"""Trainium2 Bass kernel for nn_CrossAttention (B=2, L=1024, S=2048, DIM=1024, H=16 heads).

Sharding: tensor-parallel over heads x data-parallel over batch.
Core c handles batch b = c//4 and head-group g = c%4 (4 heads = 256 of the
1024 hidden channels).  Each core computes, for its (b, g):

    QT = (Wq_g)^T x_q^T          [256, 1024]   (m on partitions)
    KT = (Wk_g)^T x_k^T          [256, 2048]
    V  = x_v Wv_g                [2048, 256]   (s on partitions)
    per head h (d=64):
        ST_h = KT_h^T' ...       S^T[s, l] = k_s . q_l   (s on partitions)
        P_h  = exp(SCALE * ST_h)            (unnormalized, s on partitions)
        [O^T_h ; sums_h] = [V_h | 1]^T @ P_h   (ones-column folds the softmax
                                                denominator into the matmul)
        XgT_h = O^T_h * (1/sums_h)          (gpsimd partition broadcast)
    out_partial = XgT^T @ Wo_g   [1024, 1024]

Host gathers: out[b] = sum_g out_partial[4b+g] + bo.

The kernel is scalar(exp)-bound in its core, so the structure minimizes
time-to-first-exp and keeps the 64-exp stream dense:
  A: xq/xk land as ONE strided DMA each (full 16-engine striping, no
     per-chunk issue pacing).  Projections run as 2-bank double passes
     (Q-mt0, K-sh0 both mt, then the first 8 ST+exp steps standalone while
     K-sh1 passes run).  PSUM->SBUF casts on the vector engine, off the
     scalar stream.
  B: V projection (2-bank accumulators, double-pass over cached xv chunks)
     interleaved with the remaining 24 lch0 ST+exp steps.
  C: O(lch0) + ST/exp(lch1).
  D: O(lch1) (PSUM banks from the shared ring, no false deps on C's tail)
     + Wo(lch0) interleaved.   E: Wo(lch1).
Output partials are written bf16 (host accumulates fp32); the softmax
reciprocal uses the fast approximate DVE op + gpsimd partition broadcast.
"""

import sys

if "/opt/trn_rl_repo" not in sys.path:
    sys.path.insert(0, "/opt/trn_rl_repo")

import numpy as np

B, L, S, C = 2, 1024, 2048, 1024
NH, D = 16, 64          # total heads, head dim
HPC = 4                 # heads per core
M = HPC * D             # 256 output channels per core
SCALE = D ** -0.5
P = 128                 # partitions
NCORES = 8
CK = C // P             # 8 c-tiles
NST = S // P            # 16 s-tiles
LCH = 512               # l-chunk
NLCH = L // LCH         # 2

_cache = {}


def _build(debug_dumps=False):
    import concourse.tile as tile
    from concourse import mybir, bacc

    f32 = mybir.dt.float32
    bf16 = mybir.dt.bfloat16

    nc = bacc.Bacc("TRN2", target_bir_lowering=False, debug=False)

    xqT = nc.dram_tensor("xqT", [C, L], bf16, kind="ExternalInput")
    xkT = nc.dram_tensor("xkT", [C, S], bf16, kind="ExternalInput")
    xvT = nc.dram_tensor("xvT", [C, S], bf16, kind="ExternalInput")
    wq = nc.dram_tensor("wq", [C, M], bf16, kind="ExternalInput")
    wk = nc.dram_tensor("wk", [C, M], bf16, kind="ExternalInput")
    wv = nc.dram_tensor("wv", [C, M], bf16, kind="ExternalInput")
    wo = nc.dram_tensor("wo", [M, C], bf16, kind="ExternalInput")
    outp = nc.dram_tensor("outp", [L, C], bf16, kind="ExternalOutput")
    if debug_dumps:
        dbg_qt = nc.dram_tensor("dbg_qt", [P, 2, L], bf16, kind="ExternalOutput")
        dbg_kt = nc.dram_tensor("dbg_kt", [P, 2, S], bf16, kind="ExternalOutput")
        dbg_vones = nc.dram_tensor("dbg_vones", [P, NST, HPC, D + 1], bf16,
                                   kind="ExternalOutput")
        dbg_pt = nc.dram_tensor("dbg_pt", [P, 2, LCH], bf16, kind="ExternalOutput")
        dbg_rc = nc.dram_tensor("dbg_rc", [1, LCH], f32, kind="ExternalOutput")
        dbg_bc = nc.dram_tensor("dbg_bc", [D, LCH], f32, kind="ExternalOutput")
        dbg_xgt = nc.dram_tensor("dbg_xgt", [P, 2, L], bf16, kind="ExternalOutput")

    with tile.TileContext(nc) as tc:
        with tc.tile_pool(name="singles", bufs=1) as singles, \
             tc.tile_pool(name="xv_pool", bufs=2) as xvp, \
             tc.tile_pool(name="pts", bufs=34) as pts, \
             tc.tile_pool(name="small", bufs=4) as small, \
             tc.tile_pool(name="obuf", bufs=8) as obuf:

            # ---- persistent SBUF ----
            wq_sb = singles.tile([P, CK, M], bf16, tag="wq")
            wk_sb = singles.tile([P, CK, M], bf16, tag="wk")
            wv_sb = singles.tile([P, CK, M], bf16, tag="wv")
            wo_sb = singles.tile([P, M // P, C], bf16, tag="wo")
            xq_sb = singles.tile([P, CK, L], bf16, tag="xq")
            xk_sb = singles.tile([P, CK, S], bf16, tag="xkc")
            SH = S // 2
            # single big strided DMAs: 16-engine striping from issue #1,
            # no per-chunk sync-queue issue pacing.
            nc.sync.dma_start(wq_sb[:], wq.rearrange("(ck p) m -> p ck m", p=P))
            HCK = CK // 2
            for h in range(2):
                nc.sync.dma_start(
                    xq_sb[:, h * HCK:(h + 1) * HCK, :],
                    xqT[h * HCK * P:(h + 1) * HCK * P, :]
                    .rearrange("(ck p) l -> p ck l", p=P))
            nc.sync.dma_start(wk_sb[:], wk.rearrange("(ck p) m -> p ck m", p=P))
            for h in range(2):
                nc.sync.dma_start(
                    xk_sb[:, h * HCK:(h + 1) * HCK, 0:SH],
                    xkT[h * HCK * P:(h + 1) * HCK * P, 0:SH]
                    .rearrange("(ck p) s -> p ck s", p=P))
            nc.sync.dma_start(xk_sb[:, :, SH:S],
                              xkT[:, SH:S].rearrange("(ck p) s -> p ck s", p=P))
            nc.sync.dma_start(wv_sb[:], wv.rearrange("(ck p) m -> p ck m", p=P))
            nc.sync.dma_start(wo_sb[:], wo.rearrange("(kt p) n -> p kt n", p=P))

            kt_sb = singles.tile([P, 2, S], bf16, tag="kt")        # [m%128, m//128, s]
            qt_sb = singles.tile([P, 2, L], bf16, tag="qt")        # [m%128, m//128, l]
            vones = singles.tile([P, NST, HPC, D + 1], bf16, tag="vones")
            xgt_sb = singles.tile([P, 2, L], bf16, tag="xgt")
            stage = singles.tile([P, D], f32, tag="stage")
            nc.vector.memset(stage[:], 1.0)
            nc.vector.tensor_copy(vones[:, :, :, D:D + 1],
                                  stage[:].rearrange("p (a b) -> p a b", a=NST)[:, :, :, None])

            # ---- step helpers ----
            def st_step(lch, pair, st):
                """ST pair matmuls + exp; returns the PT tile."""
                lsl = slice(lch * LCH, (lch + 1) * LCH)
                ssl = slice(st * P, (st + 1) * P)
                st_ps = pst.tile([P, 2, LCH], f32, tag="st", name=f"stps_{lch}_{pair}_{st}")
                nc.tensor.matmul(
                    st_ps[:, 0, :], kt_sb[0:D, pair, ssl], qt_sb[0:D, pair, lsl],
                    start=True, stop=True)
                nc.tensor.matmul(
                    st_ps[:, 1, :], kt_sb[D:P, pair, ssl], qt_sb[D:P, pair, lsl],
                    start=True, stop=True, tile_position=(64, 0))
                pt_t = pts.tile([P, 2, LCH], bf16, tag="pt", name=f"pt_{lch}_{pair}_{st}")
                nc.scalar.activation(pt_t[:], st_ps[:],
                                     mybir.ActivationFunctionType.Exp, scale=SCALE)
                if debug_dumps and lch == 0 and pair == 0 and st == 0:
                    nc.sync.dma_start(dbg_pt[:], pt_t[:])
                return pt_t

            def o_step(o_ps, lch, pair, st, pt_t):
                for hh in range(2):
                    nc.tensor.matmul(
                        o_ps[hh][:], vones[:, st, pair * 2 + hh, :], pt_t[:, hh, :],
                        start=(st == 0), stop=(st == NST - 1))

            def norm_pair(lch, pair, o_ps):
                """fast reciprocal of sums row -> gpsimd partition broadcast
                -> normalized XgT (no PSUM bank, no tensor-engine matmul)."""
                lsl = slice(lch * LCH, (lch + 1) * LCH)
                for hh in range(2):
                    # rc lives at partition 0: the gpsimd broadcast firmware
                    # reads the source on Q7 core 0, which only sees
                    # partitions 0-15.  Stage the PSUM sums row into SBUF
                    # first (custom-DVE bit ops need an SBUF source).
                    sums_sb = small.tile([1, LCH], f32, tag="sums")
                    nc.vector.tensor_copy(sums_sb[:], o_ps[hh][D:D + 1, :])
                    rc = small.tile([1, LCH], f32, tag="rc")
                    nc.vector.reciprocal_approx_fast(rc[:], sums_sb[:])
                    bc_sb = small.tile([D, LCH], f32, tag="bc")
                    nc.gpsimd.partition_broadcast(bc_sb[:], rc[:])
                    if debug_dumps and lch == 0 and pair == 0 and hh == 0:
                        nc.sync.dma_start(dbg_rc[:], rc[:])
                        nc.sync.dma_start(dbg_bc[:], bc_sb[:])
                    nc.vector.tensor_mul(
                        xgt_sb[hh * D:(hh + 1) * D, pair, lsl],
                        o_ps[hh][0:D, :], bc_sb[:])

            def wo_step(pool, lt, nch, cast_eng):
                wo_ps = pool.tile([P, 512], f32, tag="wo", name=f"wops_{lt}_{nch}")
                for kt in range(2):
                    nc.tensor.matmul(
                        wo_ps[:], xgt_sb[:, kt, lt * P:(lt + 1) * P],
                        wo_sb[:, kt, nch * 512:(nch + 1) * 512],
                        start=(kt == 0), stop=(kt == 1))
                ob_sb = obuf.tile([P, 512], bf16, tag="ob")
                if cast_eng == "scalar":
                    nc.scalar.copy(ob_sb[:], wo_ps[:])
                    nc.scalar.dma_start(
                        outp[lt * P:(lt + 1) * P, nch * 512:(nch + 1) * 512], ob_sb[:])
                else:
                    nc.vector.tensor_copy(ob_sb[:], wo_ps[:])
                    nc.gpsimd.dma_start(
                        outp[lt * P:(lt + 1) * P, nch * 512:(nch + 1) * 512], ob_sb[:])

            # ---- PSUM pool timeline (LIFO):
            #   shared(2) > pst(4) > [psp(2) A] > [ps1(2) C] > close pst >
            #   [wo(4) D/E] > close shared
            shared_cm = tc.tile_pool(name="ps_shared", bufs=2, space="PSUM")
            shared = shared_cm.__enter__()
            pst_cm = tc.tile_pool(name="ps_st", bufs=2, space="PSUM")
            pst = pst_cm.__enter__()

            pt0 = {}   # (pair, st) -> PT tile for lch 0
            pt1 = {}

            # =========== Phase A: QT + KT projections ===========
            with tc.tile_pool(name="ps_proj", bufs=2, space="PSUM") as psp:

                def q_pass(mt):
                    q_ps = [psp.tile([P, 512], f32, tag="pp", name=f"qtps{mt}_{lh}")
                            for lh in range(2)]
                    for ck in range(CK):
                        for lh in range(2):
                            nc.tensor.matmul(
                                q_ps[lh][:],
                                wq_sb[:, ck, mt * P:(mt + 1) * P],
                                xq_sb[:, ck, lh * 512:(lh + 1) * 512],
                                start=(ck == 0), stop=(ck == CK - 1))
                    for lh in range(2):
                        nc.vector.tensor_copy(
                            qt_sb[:, mt, lh * 512:(lh + 1) * 512], q_ps[lh][:])

                def k_pass(mt, sh):
                    k_ps = [psp.tile([P, 512], f32, tag="pp",
                                     name=f"ktps{sh}_{mt}_{nh}") for nh in range(2)]
                    for ck in range(CK):
                        for nh in range(2):
                            nc.tensor.matmul(
                                k_ps[nh][:],
                                wk_sb[:, ck, mt * P:(mt + 1) * P],
                                xk_sb[:, ck, sh * SH + nh * 512:sh * SH + (nh + 1) * 512],
                                start=(ck == 0), stop=(ck == CK - 1))
                    for nh in range(2):
                        nc.vector.tensor_copy(
                            kt_sb[:, mt, sh * SH + nh * 512:sh * SH + (nh + 1) * 512],
                            k_ps[nh][:])

                q_pass(0)
                k_pass(0, 0)
                # first 8 ST+exp steps run standalone: they only need the
                # mt0 projections and KT s-half 0 (s-tiles 0-7).
                for st in range(8):
                    pt0[(0, st)] = st_step(0, 0, st)
                k_pass(0, 1)
                q_pass(1)
                k_pass(1, 0)
                k_pass(1, 1)

            # =========== Phase B: V projection + remaining lch0 ST steps ====
            step = CK  # pair-0 s-tiles 0-7 ran in phase A
            for q in range(4):
                xv_t = xvp.tile([P, CK, 4 * P], bf16, tag="xv")
                nc.sync.dma_start(
                    xv_t[:], xvT[:, q * 4 * P:(q + 1) * 4 * P]
                    .rearrange("(ck p) s -> p ck s", p=P))
                for half in range(2):
                    v_ps = [shared.tile([P, M], f32, tag="sh",
                                        name=f"vps{q}_{half}_{i}")
                            for i in range(2)]
                    for ck in range(CK):
                        for st2 in range(2):
                            st4 = half * 2 + st2
                            nc.tensor.matmul(
                                v_ps[st2][:],
                                xv_t[:, ck, st4 * P:(st4 + 1) * P],
                                wv_sb[:, ck, :],
                                start=(ck == 0), stop=(ck == CK - 1))
                        if ck % 2 == half and step < 2 * NST:
                            pair, st = divmod(step, NST)
                            pt0[(pair, st)] = st_step(0, pair, st)
                            step += 1
                    for st2 in range(2):
                        st = q * 4 + half * 2 + st2
                        nc.vector.tensor_copy(
                            vones[:, st, :, 0:D],
                            v_ps[st2][:].rearrange("p (h d) -> p h d", h=HPC))

            # =========== Phase C: O(lch0) + lch1 ST/exp ===========
            with tc.tile_pool(name="ps_c", bufs=2, space="PSUM") as ps1:
                for pair in range(2):
                    o_ps = [ps1.tile([D + 1, LCH], f32, tag="ps1",
                                     name=f"ops0_{pair}_{i}") for i in range(2)]
                    for st in range(NST):
                        o_step(o_ps, 0, pair, st, pt0.pop((pair, st)))
                        pt1[(pair, st)] = st_step(1, pair, st)
                    norm_pair(0, pair, o_ps)

            pst_cm.__exit__(None, None, None)

            # =========== Phases D+E: O(lch1) + Wo ===========
            with tc.tile_pool(name="ps_wo", bufs=4, space="PSUM") as pswo:
                wo_jobs0 = [(lt, nch) for lt in range(4) for nch in range(2)]
                ncast = 0
                for pair in range(2):
                    o_ps = [shared.tile([D + 1, LCH], f32, tag="sh",
                                        name=f"ops1_{pair}_{i}") for i in range(2)]
                    for st in range(NST):
                        o_step(o_ps, 1, pair, st, pt1.pop((pair, st)))
                        if st % 4 == 1 and wo_jobs0:
                            wo_step(pswo, *wo_jobs0.pop(0),
                                    "scalar" if ncast % 2 == 0 else "vector")
                            ncast += 1
                    norm_pair(1, pair, o_ps)
                for lt, nch in wo_jobs0:
                    wo_step(pswo, lt, nch, "scalar" if ncast % 2 == 0 else "vector")
                    ncast += 1

                for lt in range(4, 8):
                    for nch in range(2):
                        wo_step(pswo, lt, nch,
                                "scalar" if ncast % 2 == 0 else "vector")
                        ncast += 1

                if debug_dumps:
                    nc.sync.dma_start(dbg_qt[:], qt_sb[:])
                    nc.sync.dma_start(dbg_kt[:], kt_sb[:])
                    nc.sync.dma_start(dbg_vones[:], vones[:])
                    nc.sync.dma_start(dbg_xgt[:], xgt_sb[:])

            shared_cm.__exit__(None, None, None)

    nc.compile()
    return nc


def _get_nc():
    if "nc" not in _cache:
        _cache["nc"] = _build()
    return _cache["nc"]


def _make_in_maps(inputs):
    import ml_dtypes

    bf16 = ml_dtypes.bfloat16
    query = np.asarray(inputs["query"], dtype=np.float32)
    key = np.asarray(inputs["key"], dtype=np.float32)
    value = np.asarray(inputs["value"], dtype=np.float32)
    Wq = np.asarray(inputs["Wq"], dtype=np.float32)
    Wk = np.asarray(inputs["Wk"], dtype=np.float32)
    Wv = np.asarray(inputs["Wv"], dtype=np.float32)
    Wo = np.asarray(inputs["Wo"], dtype=np.float32)

    qT = [np.ascontiguousarray(query[b].T).astype(bf16) for b in range(B)]
    kT = [np.ascontiguousarray(key[b].T).astype(bf16) for b in range(B)]
    vT = [np.ascontiguousarray(value[b].T).astype(bf16) for b in range(B)]
    wq_s = [np.ascontiguousarray(Wq[:, g * M:(g + 1) * M]).astype(bf16) for g in range(4)]
    wk_s = [np.ascontiguousarray(Wk[:, g * M:(g + 1) * M]).astype(bf16) for g in range(4)]
    wv_s = [np.ascontiguousarray(Wv[:, g * M:(g + 1) * M]).astype(bf16) for g in range(4)]
    wo_s = [np.ascontiguousarray(Wo[g * M:(g + 1) * M, :]).astype(bf16) for g in range(4)]

    in_maps = []
    for core in range(NCORES):
        b, g = core // 4, core % 4
        in_maps.append({
            "xqT": qT[b], "xkT": kT[b], "xvT": vT[b],
            "wq": wq_s[g], "wk": wk_s[g], "wv": wv_s[g], "wo": wo_s[g],
        })
    return in_maps


def kernel(query, key, value, Wq, Wk, Wv, Wo, bo):
    from concourse.bass_utils import run_bass_kernel_spmd

    nc = _get_nc()
    bo = np.asarray(bo, dtype=np.float32)
    in_maps = _make_in_maps(dict(query=query, key=key, value=value,
                                 Wq=Wq, Wk=Wk, Wv=Wv, Wo=Wo))

    res = run_bass_kernel_spmd(nc, in_maps, core_ids=list(range(NCORES)))

    out = np.zeros((B, L, C), dtype=np.float32)
    for core in range(NCORES):
        b = core // 4
        out[b] += np.asarray(res.results[core]["outp"], dtype=np.float32)
    out += bo[None, None, :]
    return out
